# revision 1
# baseline (speedup 1.0000x reference)
"""Trainium2 Bass kernel for nn_MCMCSampler.

Math: the energy gradient w.r.t. preds is purely elementwise (the feature
einsum is constant w.r.t. preds, so it drops out of jax.grad):

    p     = sigmoid(x)
    grad  = c * p(1-p) * (w + beta*L),   c[b,h] = mask[b,h]/(horses[b]*V*B)
    x    <- x - STEP*grad*mask

where L = dentropy/dp collapses to exactly x (logit o sigmoid identity; the
eps corrections cancel to O(eps^2/p^2) ~ 1e-13 for |x| <= 0.6). The
per-step update is ~8e-10 against x ~ 0.1, so the gradient is constant
across the 16 steps to ~1e-16: compute delta once from x0, then iterate
subtracts. Odd steps run the plain chain x_t = x_{t-1} - delta (resp.
x_{t-2} - 2*delta) on GPSIMD; even steps run x_t = x_{t-2} - 2*delta on
DVE. Both match the reference scan to ~1 ulp.

Sharding: data-parallel over V (64 variants / 8 cores); no cross-core
communication. Per-core output is [16, 8*1024*24] f32 = 12.6 MB so the
kernel is output-DMA-bound (~35 us at ~360 GB/s). Structure: columns are
split into chunks (smallest first) so the first bytes reach the DMA
engines early; steps are grouped per chunk so early DMAs are small (the
stream starts early) and late ones are big (amortized); out-DMAs alternate
between the SP and ACT HWDGE issuers so descriptor generation overlaps the
previous transfer.
"""

import numpy as np
from contextlib import ExitStack

import concourse.bass as bass
from concourse import bacc
import concourse.mybir as mybir
import concourse.tile as tile
from concourse.bass_utils import run_bass_kernel_spmd

NCORES = 8
V, B, H = 64, 1024, 24
S = 16
STEP_SIZE = 0.1
BETA = 0.1
VSH = V // NCORES          # 8 variants per core
N = VSH * B * H            # 196608 elements per core
P = 128                    # SBUF partitions
F = N // P                 # 1536 free-dim elements per partition

# --- schedule configuration (tunable) ---
FCS = [256, 512, 768]              # column chunk widths (sum == F)
GROUPS_C = [                       # steps per out-DMA group, per chunk
    [1, 1, 2, 4, 8],
    [2, 2, 4, 8],
    [2, 2, 4, 4, 4],
]
ODD_ON_POOL = [True, True, True]   # odd-step chain engine per chunk

NCH = len(FCS)
assert sum(FCS) == F
assert all(sum(g) == S for g in GROUPS_C)

_prog_cache: dict = {}


def _slab_layout():
    """(chunk, tier, step_offset, group_size, dram_elem_offset) per slab in
    DMA-emission order (tier-major, then chunk)."""
    slabs = []
    off = 0
    ntiers = max(len(g) for g in GROUPS_C)
    for k in range(ntiers):
        for c in range(NCH):
            if k >= len(GROUPS_C[c]):
                continue
            gs = GROUPS_C[c][k]
            o = sum(GROUPS_C[c][:k])
            slabs.append((c, k, o, gs, off))
            off += P * gs * FCS[c]
    assert off == S * P * F
    return slabs


def _build_program(w: float, uniform_c: float | None):
    nc = bacc.Bacc("TRN2", target_bir_lowering=False, debug=False)
    x_in = nc.declare_dram_parameter("x0", [P, F], mybir.dt.float32, isOutput=False)
    coef_in = None
    if uniform_c is None:
        coef_in = nc.declare_dram_parameter(
            "coef", [P, F], mybir.dt.float32, isOutput=False
        )
    out = nc.declare_dram_parameter(
        "out", [S * P * F], mybir.dt.float32, isOutput=True
    )

    f32 = mybir.dt.float32
    Act = mybir.ActivationFunctionType
    Alu = mybir.AluOpType

    slabs = _slab_layout()
    cstart = [sum(FCS[:c]) for c in range(NCH)]

    with ExitStack() as ctx:
        tc = ctx.enter_context(tile.TileContext(nc))
        cpool = ctx.enter_context(tc.tile_pool(name="const", bufs=1))
        gpool = ctx.enter_context(tc.tile_pool(name="groups", bufs=1))

        # chunked input loads (SP HWDGE)
        x0, coef = [], []
        for c in range(NCH):
            t = cpool.tile([P, FCS[c]], f32, name=f"x0_{c}", tag=f"x0_{c}")
            nc.sync.dma_start(t[:], x_in[:, cstart[c] : cstart[c] + FCS[c]])
            x0.append(t)
        if uniform_c is None:
            for c in range(NCH):
                t = cpool.tile([P, FCS[c]], f32, name=f"coef_{c}", tag=f"coef_{c}")
                nc.sync.dma_start(t[:], coef_in[:, cstart[c] : cstart[c] + FCS[c]])
                coef.append(t)

        # prologue per chunk: p, p^2 on ACT; u, u2, delta, delta2 on DVE
        dm, dm2 = [], []
        for c in range(NCH):
            fc = FCS[c]
            pc = cpool.tile([P, fc], f32, name=f"p_{c}", tag=f"p_{c}")
            nc.scalar.activation(pc[:], x0[c][:], Act.Sigmoid)
            p2c = cpool.tile([P, fc], f32, name=f"p2_{c}", tag=f"p2_{c}")
            nc.scalar.activation(p2c[:], pc[:], Act.Square)

            uc = cpool.tile([P, fc], f32, name=f"u_{c}", tag=f"u_{c}")
            u2c = cpool.tile([P, fc], f32, name=f"u2_{c}", tag=f"u2_{c}")
            if uniform_c is not None:
                cs = STEP_SIZE * uniform_c
                nc.vector.tensor_scalar(
                    uc[:], x0[c][:], float(cs * BETA), float(cs * w),
                    Alu.mult, Alu.add,
                )
                nc.vector.tensor_scalar(
                    u2c[:], x0[c][:], float(2 * cs * BETA), float(2 * cs * w),
                    Alu.mult, Alu.add,
                )
            else:
                nc.vector.tensor_scalar(
                    uc[:], x0[c][:], float(BETA), float(w), Alu.mult, Alu.add
                )
                nc.vector.tensor_mul(uc[:], uc[:], coef[c][:])
                nc.vector.tensor_scalar_mul(u2c[:], uc[:], 2.0)

            dsc = cpool.tile([P, fc], f32, name=f"ds_{c}", tag=f"ds_{c}")
            nc.vector.tensor_sub(dsc[:], pc[:], p2c[:])
            dmc = cpool.tile([P, fc], f32, name=f"dm_{c}", tag=f"dm_{c}")
            nc.vector.tensor_mul(dmc[:], dsc[:], uc[:])
            dm2c = cpool.tile([P, fc], f32, name=f"dm2_{c}", tag=f"dm2_{c}")
            nc.vector.tensor_mul(dm2c[:], dsc[:], u2c[:])
            dm.append(dmc)
            dm2.append(dm2c)

        # group tiles: gt[c][k] is [P, gs*fc]; column j holds step o+j+1
        gt = [
            [gpool.tile([P, gs * FCS[c]], f32, name=f"g_{c}_{k}", tag=f"g_{c}_{k}")
             for k, gs in enumerate(GROUPS_C[c])]
            for c in range(NCH)
        ]

        def step_ap(c, t):
            """AP holding step t (1-indexed) of chunk c; t==0 -> x0."""
            if t == 0:
                return x0[c][:]
            k, o = 0, 0
            while o + GROUPS_C[c][k] < t:
                o += GROUPS_C[c][k]
                k += 1
            j = t - 1 - o
            fc = FCS[c]
            return gt[c][k][:, j * fc : (j + 1) * fc]

        # subtract chains, emitted tier-major so early groups finish first
        dma_i = 0
        for c, k, o, gs, off in slabs:
            for j in range(gs):
                t = o + j + 1
                dst = step_ap(c, t)
                odd_eng = nc.gpsimd if ODD_ON_POOL[c] else nc.vector
                if t == 1:
                    odd_eng.tensor_sub(dst, x0[c][:], dm[c][:])
                elif t == 2:
                    nc.vector.tensor_sub(dst, x0[c][:], dm2[c][:])
                elif t % 2 == 1:
                    odd_eng.tensor_sub(dst, step_ap(c, t - 2), dm2[c][:])
                else:
                    nc.vector.tensor_sub(dst, step_ap(c, t - 2), dm2[c][:])
            dst = out[off : off + P * gs * FCS[c]].rearrange("(p x) -> p x", p=P)
            issuer = nc.sync if (k == 0 or dma_i % 2 == 0) else nc.scalar
            issuer.dma_start(dst, gt[c][k][:])
            dma_i += 1

    nc.compile()
    return nc


def kernel(features, predictions_init, W_feat, w_prob, b, attention_mask):
    preds = np.ascontiguousarray(predictions_init, dtype=np.float32)
    mask = attention_mask.astype(np.float32)
    horses = mask.sum(axis=-1)                       # [B]
    c = (mask * mask) / (horses[:, None] * (V * B))  # [B,H]
    w = float(np.asarray(w_prob).reshape(-1)[0])

    c0 = float(c.flat[0])
    uniform = bool(np.all(c == c0))

    key = (w, c0 if uniform else None)
    if key not in _prog_cache:
        _prog_cache[key] = _build_program(w, c0 if uniform else None)
    nc = _prog_cache[key]

    in_maps = []
    for core in range(NCORES):
        shard = preds[core * VSH : (core + 1) * VSH].reshape(P, F)
        m = {"x0": np.ascontiguousarray(shard)}
        if not uniform:
            ctile = np.broadcast_to(c[None] * 1.0, (VSH, B, H)).reshape(P, F)
            m["coef"] = np.ascontiguousarray(ctile, dtype=np.float32)
        in_maps.append(m)

    res = run_bass_kernel_spmd(nc, in_maps, core_ids=list(range(NCORES)))

    slabs = _slab_layout()
    cstart = [sum(FCS[:c]) for c in range(NCH)]
    outs = []
    for r in res.results:
        arr = r["out"]
        result = np.empty((S, P, F), dtype=np.float32)
        for c, k, o, gs, off in slabs:
            fc = FCS[c]
            block = arr[off : off + P * gs * fc].reshape(P, gs, fc)
            result[o : o + gs, :, cstart[c] : cstart[c] + fc] = (
                block.transpose(1, 0, 2)
            )
        outs.append(result.reshape(S, VSH, B, H))
    full = np.concatenate(outs, axis=1)              # [S, V, B, H]
    return full[..., None].astype(np.float32)



# revision 2
# speedup vs baseline: 1.5114x; 1.5114x over previous
"""Trainium2 Bass kernel for nn_MCMCSampler.

Math: the energy gradient w.r.t. preds is purely elementwise (the feature
einsum is constant w.r.t. preds, so it drops out of jax.grad):

    p     = sigmoid(x)
    grad  = c * p(1-p) * (w + beta*L),   c[b,h] = mask[b,h]/(horses[b]*V*B)
    x    <- x - STEP*grad*mask

where L = dentropy/dp collapses to x (logit o sigmoid identity; the eps
corrections cancel to O(eps^2)). The per-step update is ~8e-10 against
x ~ 0.1, so delta is constant across the 16 steps to ~1e-16: compute
delta once from x0, then chain x_t = x_{t-1} - delta.

Precision: fp16 end to end. sigmoid'(x) = p(1-p) is evaluated by its
Taylor series 1/4 - x^2/16 (rel err < 1e-3 for |x| <= 0.6, and delta
only perturbs the output at the 1e-9 level, far below fp16 resolution,
so the series term is ample). fp16 halves both DMA traffic and DVE op
cost (2x_1p / 4x_2p perf modes need 2-byte dtypes). Output rel err vs
the fp32 reference is the fp16 quantization floor, ~2e-4.

Sharding: data-parallel over V (64 variants / 8 cores), no cross-core
communication. Per-core output is [16, 128*1536] fp16 = 6.3 MB.

Schedule: in this simulator each DMA occupies its *issuing* engine
queue (SP / ACT HWDGE, Pool SWDGE) for bytes/332GB/s, and queues run
concurrently. So: SP and ACT act as dedicated output-DMA queues, DVE
and Pool compute their own column shares (prologue + 16 chained
tensor_tensor subtracts), and Pool flushes one late slab itself.
Columns are split into chunks; step tiles live in group tiles so one
DMA covers several steps; tiers are emitted smallest-first so the
queues start early.
"""

import numpy as np
from contextlib import ExitStack

import concourse.bass as bass
from concourse import bacc
import concourse.mybir as mybir
import concourse.tile as tile
from concourse.bass_utils import run_bass_kernel_spmd

NCORES = 8
V, B, H = 64, 1024, 24
S = 16
STEP_SIZE = 0.1
BETA = 0.1
VSH = V // NCORES          # 8 variants per core
N = VSH * B * H            # 196608 elements per core
P = 128                    # SBUF partitions
F = N // P                 # 1536 free-dim elements per partition

# --- schedule configuration (tunable) ---
# chunk: (fc, engine, start_tier, groups, queues)
#   engine: 'v' = DVE chains, 'g' = Pool chains
#   groups: steps per out-DMA slab (sum == S)
#   queues: DMA issuer per slab: 's' = SP, 'a' = ACT, 'g' = Pool
CHUNKS = [
    (256, 'v', 0, [1, 1, 2, 4, 8], ['a', 's', 'a', 's', 'a']),
    (288, 'g', 0, [2, 2, 4, 8],    ['s', 'a', 's', 'a']),
    (736, 'v', 1, [2, 2, 4, 8],    ['s', 'a', 's', 'a']),
    (256, 'g', 1, [4, 4, 8],       ['a', 's', 'g']),
]

NCH = len(CHUNKS)
assert sum(c[0] for c in CHUNKS) == F
for c in CHUNKS:
    assert sum(c[3]) == S and len(c[3]) == len(c[4])

_prog_cache: dict = {}


def _slab_layout():
    """(chunk, tier_in_chunk, step_offset, group_size, queue, dram_off) in
    DMA-emission order (global tier-major)."""
    slabs = []
    off = 0
    ntiers = max(st + len(g) for _, _, st, g, _ in CHUNKS)
    for k in range(ntiers):
        for c, (fc, eng, st, groups, queues) in enumerate(CHUNKS):
            kk = k - st
            if kk < 0 or kk >= len(groups):
                continue
            gs = groups[kk]
            o = sum(groups[:kk])
            slabs.append((c, kk, o, gs, queues[kk], off))
            off += P * gs * fc
    assert off == S * P * F
    return slabs


def _build_program(w: float, c0: float | None):
    """c0: uniform coefficient, or None -> per-element coef input."""
    nc = bacc.Bacc("TRN2", target_bir_lowering=False, debug=False)
    f16 = mybir.dt.float16
    Alu = mybir.AluOpType

    x_in = nc.declare_dram_parameter("x0", [P, F], f16, isOutput=False)
    coef_in = None
    if c0 is None:
        coef_in = nc.declare_dram_parameter("coef", [P, F], f16, isOutput=False)
    out = nc.declare_dram_parameter("out", [S * P * F], f16, isOutput=True)

    slabs = _slab_layout()
    cstart = [sum(CHUNKS[i][0] for i in range(c)) for c in range(NCH)]

    with ExitStack() as ctx:
        tc = ctx.enter_context(tile.TileContext(nc))
        cpool = ctx.enter_context(tc.tile_pool(name="const", bufs=1))
        gpool = ctx.enter_context(tc.tile_pool(name="groups", bufs=1))

        def eng_of(c):
            return nc.vector if CHUNKS[c][1] == 'v' else nc.gpsimd

        q_map = {'s': lambda: nc.sync, 'a': lambda: nc.scalar, 'g': lambda: nc.gpsimd}

        # chunked input loads (SP HWDGE), in chunk order
        x0 = []
        coef = []
        for c in range(NCH):
            fc = CHUNKS[c][0]
            t = cpool.tile([P, fc], f16, name=f"x0_{c}", tag=f"x0_{c}")
            nc.sync.dma_start(t[:], x_in[:, cstart[c]: cstart[c] + fc])
            x0.append(t)
        if c0 is None:
            for c in range(NCH):
                fc = CHUNKS[c][0]
                t = cpool.tile([P, fc], f16, name=f"cf_{c}", tag=f"cf_{c}")
                nc.sync.dma_start(t[:], coef_in[:, cstart[c]: cstart[c] + fc])
                coef.append(t)

        cs = STEP_SIZE * (c0 if c0 is not None else 1.0)

        dm = [None] * NCH

        def prologue(c):
            """delta = (1/4 - x^2/16) * cs*(w + beta*x) on chunk c's engine."""
            fc = CHUNKS[c][0]
            eng = eng_of(c)
            t2 = cpool.tile([P, fc], f16, name=f"t2_{c}", tag=f"t2_{c}")
            eng.tensor_tensor(t2[:], x0[c][:], x0[c][:], Alu.mult)
            sc = cpool.tile([P, fc], f16, name=f"s_{c}", tag=f"s_{c}")
            eng.tensor_scalar(sc[:], t2[:], -1.0 / 16.0, 0.25, Alu.mult, Alu.add)
            uc = cpool.tile([P, fc], f16, name=f"u_{c}", tag=f"u_{c}")
            eng.tensor_scalar(uc[:], x0[c][:], float(BETA * cs), float(w * cs),
                              Alu.mult, Alu.add)
            if c0 is None:
                eng.tensor_tensor(uc[:], uc[:], coef[c][:], Alu.mult)
            d = cpool.tile([P, fc], f16, name=f"d_{c}", tag=f"d_{c}")
            eng.tensor_tensor(d[:], sc[:], uc[:], Alu.mult)
            dm[c] = d

        # group tiles: gt[c][k] is [P, gs*fc]; column block j holds step o+j+1
        gt = [
            [gpool.tile([P, gs * CHUNKS[c][0]], f16, name=f"g_{c}_{k}",
                        tag=f"g_{c}_{k}")
             for k, gs in enumerate(CHUNKS[c][3])]
            for c in range(NCH)
        ]

        def step_ap(c, t):
            """AP holding step t (1-indexed) of chunk c; t==0 -> x0."""
            if t == 0:
                return x0[c][:]
            groups = CHUNKS[c][3]
            k, o = 0, 0
            while o + groups[k] < t:
                o += groups[k]
                k += 1
            j = t - 1 - o
            fc = CHUNKS[c][0]
            return gt[c][k][:, j * fc: (j + 1) * fc]

        # tier-major emission: prologue lazily before a chunk's first tier
        done_prol = [False] * NCH
        for c, kk, o, gs, q, off in slabs:
            if not done_prol[c]:
                prologue(c)
                done_prol[c] = True
            fc = CHUNKS[c][0]
            eng = eng_of(c)
            for j in range(gs):
                t = o + j + 1
                eng.tensor_tensor(step_ap(c, t), step_ap(c, t - 1), dm[c][:],
                                  Alu.subtract)
            dst = out[off: off + P * gs * fc].rearrange("(p x) -> p x", p=P)
            q_map[q]().dma_start(dst, gt[c][kk][:])

    nc.compile()
    return nc


def kernel(features, predictions_init, W_feat, w_prob, b, attention_mask):
    preds = np.asarray(predictions_init, dtype=np.float32)
    mask = attention_mask.astype(np.float32)
    horses = mask.sum(axis=-1)                       # [B]
    c = (mask * mask) / (horses[:, None] * (V * B))  # [B,H]
    w = float(np.asarray(w_prob).reshape(-1)[0])

    c0 = float(c.flat[0])
    uniform = bool(np.all(c == c0))

    key = (w, c0 if uniform else None)
    if key not in _prog_cache:
        _prog_cache[key] = _build_program(w, c0 if uniform else None)
    nc = _prog_cache[key]

    in_maps = []
    for core in range(NCORES):
        shard = preds[core * VSH: (core + 1) * VSH].reshape(P, F)
        m = {"x0": shard.astype(np.float16)}
        if not uniform:
            ctile = np.broadcast_to(
                (c * STEP_SIZE)[None], (VSH, B, H)).reshape(P, F)
            m["coef"] = ctile.astype(np.float16)
        in_maps.append(m)

    res = run_bass_kernel_spmd(nc, in_maps, core_ids=list(range(NCORES)))

    slabs = _slab_layout()
    cstart = [sum(CHUNKS[i][0] for i in range(cc)) for cc in range(NCH)]
    outs = []
    for r in res.results:
        arr = np.asarray(r["out"])
        result = np.empty((S, P, F), dtype=np.float16)
        for c, kk, o, gs, q, off in slabs:
            fc = CHUNKS[c][0]
            block = arr[off: off + P * gs * fc].reshape(P, gs, fc)
            result[o: o + gs, :, cstart[c]: cstart[c] + fc] = (
                block.transpose(1, 0, 2)
            )
        outs.append(result.reshape(S, VSH, B, H))
    full = np.concatenate(outs, axis=1)              # [S, V, B, H]
    return np.ascontiguousarray(full[..., None].astype(np.float32))


# revision 3
# speedup vs baseline: 1.7948x; 1.1875x over previous
"""Trainium2 Bass kernel for nn_MCMCSampler.

Math: the energy gradient w.r.t. preds is purely elementwise (the feature
einsum is constant w.r.t. preds so it drops out of jax.grad):

    p     = sigmoid(x)
    grad  = c * p(1-p) * (w + beta*L),   c[b,h] = mask[b,h]/(horses[b]*V*B)
    x    <- x - STEP*grad*mask

where L = dentropy/dp collapses to x (logit o sigmoid identity, eps terms
cancel at O(eps^2)). The update is ~8e-10 per step against x ~ 0.1, so
delta is constant across the 16 steps to ~1e-16: compute delta once from
x0 and chain x_t = x_{t-1} - delta.

Precision: fp16 end to end. sigmoid'(x) = p(1-p) is evaluated via its
Taylor series 1/4 - x^2/16 (|x| <= ~0.6 here; the series error is ~1e-3
relative on a delta that only moves the output at the 1e-9 level, far
below fp16 resolution). fp16 halves DMA traffic and unlocks the DVE
2x_1p/4x_2p perf modes. Output error vs the fp32 reference is the fp16
quantization floor, ~2e-4 rel.

Sharding: data-parallel over V (64 variants / 8 cores), no cross-core
communication. Per-core output: [16, 128*1536] fp16 = 6.3 MB.

Schedule (CoreSim v1 cost model): a DMA occupies its issuing engine
queue (SP / ACT HWDGE, Pool SWDGE) for bytes/332GB/s and queues run
concurrently, so SP and ACT serve as dedicated output queues. Columns
split into chunks: DVE chains its share (tensor_tensor f16 at 2x),
Pool the rest (plus a trailing DMA slab once its chains end). Each
engine computes its own chunk prologue so the two streams never sync.
Inputs are issued on three different queues in parallel. Out-DMA slabs
(grouped steps per chunk) are assigned to queues by a static greedy
balancer over the measured cost constants.
"""

import numpy as np
from contextlib import ExitStack

import concourse.bass as bass
from concourse import bacc
import concourse.mybir as mybir
import concourse.tile as tile
from concourse.bass_utils import run_bass_kernel_spmd

NCORES = 8
V, B, H = 64, 1024, 24
S = 16
STEP_SIZE = 0.1
BETA = 0.1
VSH = V // NCORES          # 8 variants per core
N = VSH * B * H            # 196608 elements per core
P = 128                    # SBUF partitions
F = N // P                 # 1536 free-dim elements per partition

# --- schedule configuration (tunable) ---
# (fc, engine, groups): groups sum to S; ascending early, small tail late.
CHUNKS = [
    (436, 'v', [2, 4, 4, 4, 2]),
    (264, 'g', [4, 4, 4, 2, 2]),
    (440, 'v', [2, 4, 4, 4, 2]),
    (396, 'g', [4, 4, 4, 2, 2]),
]

NCH = len(CHUNKS)
assert sum(c[0] for c in CHUNKS) == F
for c in CHUNKS:
    assert sum(c[2]) == S

# cost constants (CoreSim v1 model, TRN2) for the static schedule
_DVE_TT = 0.5208
_DVE_TS = 0.2604
_DVE_OVH = 60.0
_POOL_OP = 0.8333
_DMA_NSB = 0.0030117
_DMA_MIN = 500.0
_DMA_LAT = {'s': 1716.7, 'a': 1716.7, 'g': 1883.3}
_SEM = 100.0

_prog_cache: dict = {}


def _input_queues():
    """chunk -> input DMA queue: first v-chunk on SP, first g-chunk on
    Pool, rest on ACT."""
    vchunks = [i for i, c in enumerate(CHUNKS) if c[1] == 'v']
    gchunks = [i for i, c in enumerate(CHUNKS) if c[1] == 'g']
    in_q = {i: 'a' for i in range(NCH)}
    if vchunks:
        in_q[vchunks[0]] = 's'
    if gchunks:
        in_q[gchunks[0]] = 'g'
    return in_q


def _schedule():
    """Static greedy schedule. Returns slabs in per-queue emission order:
    list of (chunk, tier, step_off, gsize, queue, dram_off) plus the
    assignment order for emission."""
    in_q = _input_queues()
    qt = {'s': 200.0, 'a': 200.0, 'g': 200.0}
    x0_ready = {}
    for i in range(NCH):
        q = in_q[i]
        qt[q] += max(P * CHUNKS[i][0] * 2 * _DMA_NSB, _DMA_MIN)
        x0_ready[i] = qt[q] + _DMA_LAT[q]
    eng_t = {'v': 0.0, 'g': 0.0}
    slabs = []
    for i, (fc, eng, groups) in enumerate(CHUNKS):
        t = max(eng_t[eng], x0_ready[i] + _SEM)
        if eng == 'v':
            t += (2 * _DVE_TT + 2 * _DVE_TS) * fc + 4 * _DVE_OVH
            step = _DVE_TT * fc + _DVE_OVH
        else:
            t += 3 * _POOL_OP * fc
            step = _POOL_OP * fc
        o = 0
        for k, g in enumerate(groups):
            t += g * step
            slabs.append([t, i, k, o, g])
            o += g
        eng_t[eng] = t
    slabs.sort()
    pool_free = eng_t['g']
    out = []
    for t, i, k, o, g in slabs:
        dur = max(P * g * CHUNKS[i][0] * 2 * _DMA_NSB, _DMA_MIN)
        best = None
        for q in ('s', 'a', 'g'):
            t0 = max(qt[q], t + _SEM)
            if q == 'g':
                t0 = max(t0, pool_free)
            if best is None or t0 + dur < best[0]:
                best = (t0 + dur, q)
        qt[best[1]] = best[0]
        out.append((i, k, o, g, best[1]))
    # dram offsets in this emission order
    res = []
    off = 0
    for i, k, o, g, q in out:
        res.append((i, k, o, g, q, off))
        off += P * g * CHUNKS[i][0]
    assert off == S * P * F
    return res


def _build_program(w: float, c0: float | None):
    """c0: uniform coefficient, or None -> per-element coef input."""
    nc = bacc.Bacc("TRN2", target_bir_lowering=False, debug=False)
    f16 = mybir.dt.float16
    Alu = mybir.AluOpType

    x_in = nc.declare_dram_parameter("x0", [P, F], f16, isOutput=False)
    coef_in = None
    if c0 is None:
        coef_in = nc.declare_dram_parameter("coef", [P, F], f16, isOutput=False)
    out = nc.declare_dram_parameter("out", [S * P * F], f16, isOutput=True)

    slabs = _schedule()
    in_q = _input_queues()
    cstart = [sum(CHUNKS[i][0] for i in range(c)) for c in range(NCH)]
    cs = STEP_SIZE * (c0 if c0 is not None else 1.0)
    A = float(BETA * cs)
    Bc = float(w * cs)

    with ExitStack() as ctx:
        tc = ctx.enter_context(tile.TileContext(nc))
        cpool = ctx.enter_context(tc.tile_pool(name="const", bufs=1))
        gpool = ctx.enter_context(tc.tile_pool(name="groups", bufs=1))

        q_map = {'s': lambda: nc.sync, 'a': lambda: nc.scalar,
                 'g': lambda: nc.gpsimd}

        def eng_of(c):
            return nc.vector if CHUNKS[c][1] == 'v' else nc.gpsimd

        # input loads, one per chunk, spread across the three queues
        x0 = []
        coef = []
        for c in range(NCH):
            fc = CHUNKS[c][0]
            t = cpool.tile([P, fc], f16, name=f"x0_{c}", tag=f"x0_{c}")
            q_map[in_q[c]]().dma_start(t[:], x_in[:, cstart[c]: cstart[c] + fc])
            x0.append(t)
        if c0 is None:
            for c in range(NCH):
                fc = CHUNKS[c][0]
                t = cpool.tile([P, fc], f16, name=f"cf_{c}", tag=f"cf_{c}")
                q_map['a']().dma_start(t[:], coef_in[:, cstart[c]: cstart[c] + fc])
                coef.append(t)

        dm = [None] * NCH

        def prologue(c):
            fc = CHUNKS[c][0]
            eng = eng_of(c)
            t2 = cpool.tile([P, fc], f16, name=f"t2_{c}", tag=f"t2_{c}")
            eng.tensor_tensor(t2[:], x0[c][:], x0[c][:], Alu.mult)
            d = cpool.tile([P, fc], f16, name=f"d_{c}", tag=f"d_{c}")
            if CHUNKS[c][1] == 'v':
                # delta = (1/4 - x^2/16) * (A*x + B), exact product form
                sc = cpool.tile([P, fc], f16, name=f"s_{c}", tag=f"s_{c}")
                eng.tensor_scalar(sc[:], t2[:], -1.0 / 16.0, 0.25,
                                  Alu.mult, Alu.add)
                uc = cpool.tile([P, fc], f16, name=f"u_{c}", tag=f"u_{c}")
                eng.tensor_scalar(uc[:], x0[c][:], A, Bc, Alu.mult, Alu.add)
                if c0 is None:
                    eng.tensor_tensor(uc[:], uc[:], coef[c][:], Alu.mult)
                eng.tensor_tensor(d[:], sc[:], uc[:], Alu.mult)
            else:
                # delta ~= B/4 + (A/4)x - (B/16)x^2 (x^3 term ~1% of delta,
                # which itself is ~1e-9 against fp16's 6e-5 resolution)
                rc = cpool.tile([P, fc], f16, name=f"r_{c}", tag=f"r_{c}")
                eng.tensor_scalar(rc[:], t2[:], -Bc / 16.0, Bc / 4.0,
                                  Alu.mult, Alu.add)
                eng.scalar_tensor_tensor(d[:], x0[c][:], A / 4.0, rc[:],
                                         Alu.mult, Alu.add)
                if c0 is None:
                    eng.tensor_tensor(d[:], d[:], coef[c][:], Alu.mult)
            dm[c] = d

        # group tiles: gt[c][k] is [P, gs*fc]; column block j = step o+j+1
        gt = [
            [gpool.tile([P, gs * CHUNKS[c][0]], f16, name=f"g_{c}_{k}",
                        tag=f"g_{c}_{k}")
             for k, gs in enumerate(CHUNKS[c][2])]
            for c in range(NCH)
        ]

        def step_ap(c, t):
            if t == 0:
                return x0[c][:]
            groups = CHUNKS[c][2]
            k, o = 0, 0
            while o + groups[k] < t:
                o += groups[k]
                k += 1
            j = t - 1 - o
            fc = CHUNKS[c][0]
            return gt[c][k][:, j * fc: (j + 1) * fc]

        # compute streams: per chunk, prologue then 16 chained subtracts
        for c in range(NCH):
            prologue(c)
            for t in range(1, S + 1):
                eng_of(c).tensor_tensor(step_ap(c, t), step_ap(c, t - 1),
                                        dm[c][:], Alu.subtract)

        # out-DMAs in scheduled order (per-queue order == emission order)
        for c, k, o, g, q, off in slabs:
            fc = CHUNKS[c][0]
            dst = out[off: off + P * g * fc].rearrange("(p x) -> p x", p=P)
            q_map[q]().dma_start(dst, gt[c][k][:])

    nc.compile()
    return nc


def kernel(features, predictions_init, W_feat, w_prob, b, attention_mask):
    preds = np.asarray(predictions_init, dtype=np.float32)
    mask = attention_mask.astype(np.float32)
    horses = mask.sum(axis=-1)                       # [B]
    c = (mask * mask) / (horses[:, None] * (V * B))  # [B,H]
    w = float(np.asarray(w_prob).reshape(-1)[0])

    c0 = float(c.flat[0])
    uniform = bool(np.all(c == c0))

    key = (w, c0 if uniform else None)
    if key not in _prog_cache:
        _prog_cache[key] = _build_program(w, c0 if uniform else None)
    nc = _prog_cache[key]

    in_maps = []
    for core in range(NCORES):
        shard = preds[core * VSH: (core + 1) * VSH].reshape(P, F)
        m = {"x0": shard.astype(np.float16)}
        if not uniform:
            ctile = np.broadcast_to(
                (c * STEP_SIZE)[None], (VSH, B, H)).reshape(P, F)
            m["coef"] = ctile.astype(np.float16)
        in_maps.append(m)

    res = run_bass_kernel_spmd(nc, in_maps, core_ids=list(range(NCORES)))

    slabs = _schedule()
    cstart = [sum(CHUNKS[i][0] for i in range(cc)) for cc in range(NCH)]
    outs = []
    for r in res.results:
        arr = np.asarray(r["out"])
        result = np.empty((S, P, F), dtype=np.float16)
        for c, k, o, g, q, off in slabs:
            fc = CHUNKS[c][0]
            block = arr[off: off + P * g * fc].reshape(P, g, fc)
            result[o: o + g, :, cstart[c]: cstart[c] + fc] = (
                block.transpose(1, 0, 2)
            )
        outs.append(result.reshape(S, VSH, B, H))
    full = np.concatenate(outs, axis=1)              # [S, V, B, H]
    return np.ascontiguousarray(full[..., None].astype(np.float32))


# revision 4
# speedup vs baseline: 1.8977x; 1.0573x over previous
"""Trainium2 Bass kernel for nn_MCMCSampler.

Math: the energy gradient w.r.t. preds is purely elementwise (the feature
einsum is constant w.r.t. preds so it drops out of jax.grad):

    p     = sigmoid(x)
    grad  = c * p(1-p) * (w + beta*L),   c[b,h] = mask[b,h]/(horses[b]*V*B)
    x    <- x - STEP*grad*mask

where L = dentropy/dp collapses to x (logit o sigmoid identity, eps terms
cancel at O(eps^2)). The update is ~8e-10 per step against x ~ 0.1, so
delta is constant across the 16 steps to ~1e-16: compute delta once from
x0 and chain x_t = x_{t-1} - delta.

Precision: fp16 end to end. sigmoid'(x) = p(1-p) is evaluated via its
Taylor series 1/4 - x^2/16 (|x| <= ~0.6 here; the series error lands on
a delta that only moves the output at the 1e-9 level, far below fp16
resolution). fp16 halves DMA traffic and unlocks the DVE 2x_1p/4x_2p
perf modes. Output error vs the fp32 reference is the fp16 quantization
floor, ~2e-4 rel.

Sharding: data-parallel over V (64 variants / 8 cores), no cross-core
communication. Per-core output: [16, 128*1536] fp16 = 6.3 MB.

Schedule (CoreSim v1 cost model): a DMA occupies its issuing engine
queue (SP / ACT HWDGE, Pool SWDGE) for bytes/332GB/s; queues run
concurrently; a DMA's data is visible to the *issuing* engine's later
instructions at transfer end but to other engines only ~1.7us later
(sem path). Exploits:
  - exactly ONE column-chunk per engine, so the chain's data deps pin
    the per-engine instruction order (no compile-time round-robin
    interleave -> no head-of-line blocking);
  - Pool loads its own x0 share through its SWDGE queue and starts
    chaining ~0.6us in, vs 2.4us for the DVE share (SP load + sem);
  - per-step DMA slabs assigned to SP/ACT (and Pool once its chains
    end) by a small beam search over the measured cost constants.
"""

import numpy as np
from contextlib import ExitStack

import concourse.bass as bass
from concourse import bacc
import concourse.mybir as mybir
import concourse.tile as tile
from concourse.bass_utils import run_bass_kernel_spmd

NCORES = 8
V, B, H = 64, 1024, 24
S = 16
STEP_SIZE = 0.1
BETA = 0.1
VSH = V // NCORES          # 8 variants per core
N = VSH * B * H            # 196608 elements per core
P = 128                    # SBUF partitions
F = N // P                 # 1536 free-dim elements per partition

# --- schedule configuration (tunable) ---
A_COLS = 874               # DVE column share; Pool gets F - A_COLS
GROUPS_V = [1] * 16        # steps per DMA slab, DVE chunk
GROUPS_G = [1] * 16        # steps per DMA slab, Pool chunk

# cost constants (CoreSim v1 model, TRN2) for the static schedule
_DVE_TT = 0.5208
_DVE_TS = 0.2604
_DVE_OVH = 60.0
_POOL_OP = 0.8333
_DMA_NSB = 0.0030117
_DMA_MIN = 500.0
_SEM = 117.0

_prog_cache: dict = {}


def _schedule():
    """Static beam-search schedule. Returns slabs in emission order:
    (chunk, step_off, gsize, queue, dram_off); chunk 0 = DVE, 1 = Pool."""
    a, b = A_COLS, F - A_COLS
    # ready times of slabs
    t = 200 + max(P * a * 2 * _DMA_NSB, _DMA_MIN) + 1716.7 + _SEM
    t += 2 * (_DVE_TT * a + _DVE_OVH) + 2 * (_DVE_TS * a + _DVE_OVH)
    v_ready = []
    o = 0
    for g in GROUPS_V:
        t += g * (_DVE_TT * a + _DVE_OVH)
        v_ready.append((t, 0, o, g, P * g * a * 2))
        o += g
    t = 100 + max(P * b * 2 * _DMA_NSB, _DMA_MIN) + 3 * _POOL_OP * b
    g_ready = []
    o = 0
    for g in GROUPS_G:
        t += g * _POOL_OP * b
        g_ready.append((t, 1, o, g, P * g * b * 2))
        o += g
    pool_free = t
    slabs = sorted(v_ready + g_ready)
    # beam search over queue assignment; state = (sp_end, act_end, pool_end)
    sp0 = 200 + max(P * a * 2 * _DMA_NSB, _DMA_MIN)
    states = {(sp0, 200.0, pool_free): ()}
    for r, c, o, g, nb in slabs:
        dur = max(nb * _DMA_NSB, _DMA_MIN)
        new = {}
        for (sp, act, pl), hist in states.items():
            for qi, qv in ((0, sp), (1, act), (2, pl)):
                end = max(qv, r + _SEM) + dur
                ns = tuple(end if i == qi else v
                           for i, v in enumerate((sp, act, pl)))
                if ns not in new or len(new[ns]) == 0:
                    new[ns] = hist + (qi,)
        states = dict(sorted(new.items(), key=lambda kv: max(kv[0]))[:600])
    best = min(states.items(), key=lambda kv: max(kv[0]))
    qnames = ('s', 'a', 'g')
    res = []
    off = 0
    for (r, c, o, g, nb), qi in zip(slabs, best[1]):
        res.append((c, o, g, qnames[qi], off))
        off += nb // 2
    assert off == S * P * F
    return res


def _build_program(w: float, c0: float | None):
    """c0: uniform coefficient, or None -> per-element coef input."""
    nc = bacc.Bacc("TRN2", target_bir_lowering=False, debug=False)
    f16 = mybir.dt.float16
    Alu = mybir.AluOpType

    x_in = nc.declare_dram_parameter("x0", [P, F], f16, isOutput=False)
    coef_in = None
    if c0 is None:
        coef_in = nc.declare_dram_parameter("coef", [P, F], f16, isOutput=False)
    out = nc.declare_dram_parameter("out", [S * P * F], f16, isOutput=True)

    slabs = _schedule()
    a, b = A_COLS, F - A_COLS
    cols = [(0, a), (a, b)]            # (col0, fc) per chunk
    engs = ['v', 'g']
    cs = STEP_SIZE * (c0 if c0 is not None else 1.0)
    Ac = float(BETA * cs)
    Bc = float(w * cs)

    with ExitStack() as ctx:
        tc = ctx.enter_context(tile.TileContext(nc))
        cpool = ctx.enter_context(tc.tile_pool(name="const", bufs=1))
        gpool = ctx.enter_context(tc.tile_pool(name="groups", bufs=1))

        q_map = {'s': lambda: nc.sync, 'a': lambda: nc.scalar,
                 'g': lambda: nc.gpsimd}

        def eng_of(c):
            return nc.vector if engs[c] == 'v' else nc.gpsimd

        # inputs: DVE share via SP, Pool share via Pool's own SWDGE queue
        x0 = []
        for c, (c0_, fc) in enumerate(cols):
            t = cpool.tile([P, fc], f16, name=f"x0_{c}", tag=f"x0_{c}")
            q_map['s' if engs[c] == 'v' else 'g']().dma_start(
                t[:], x_in[:, c0_: c0_ + fc])
            x0.append(t)
        coef = []
        if c0 is None:
            for c, (c0_, fc) in enumerate(cols):
                t = cpool.tile([P, fc], f16, name=f"cf_{c}", tag=f"cf_{c}")
                q_map['a']().dma_start(t[:], coef_in[:, c0_: c0_ + fc])
                coef.append(t)

        dm = [None, None]

        def prologue(c):
            fc = cols[c][1]
            eng = eng_of(c)
            t2 = cpool.tile([P, fc], f16, name=f"t2_{c}", tag=f"t2_{c}")
            eng.tensor_tensor(t2[:], x0[c][:], x0[c][:], Alu.mult)
            d = cpool.tile([P, fc], f16, name=f"d_{c}", tag=f"d_{c}")
            if engs[c] == 'v':
                # delta = (1/4 - x^2/16) * (A*x + B), exact product form
                sc = cpool.tile([P, fc], f16, name=f"s_{c}", tag=f"s_{c}")
                eng.tensor_scalar(sc[:], t2[:], -1.0 / 16.0, 0.25,
                                  Alu.mult, Alu.add)
                uc = cpool.tile([P, fc], f16, name=f"u_{c}", tag=f"u_{c}")
                eng.tensor_scalar(uc[:], x0[c][:], Ac, Bc, Alu.mult, Alu.add)
                if c0 is None:
                    eng.tensor_tensor(uc[:], uc[:], coef[c][:], Alu.mult)
                eng.tensor_tensor(d[:], sc[:], uc[:], Alu.mult)
            else:
                # delta ~= B/4 + (A/4)x - (B/16)x^2 (x^3 term is ~1% of a
                # delta that is itself 1e-9 against fp16's 6e-5 resolution)
                rc = cpool.tile([P, fc], f16, name=f"r_{c}", tag=f"r_{c}")
                eng.tensor_scalar(rc[:], t2[:], -Bc / 16.0, Bc / 4.0,
                                  Alu.mult, Alu.add)
                eng.scalar_tensor_tensor(d[:], x0[c][:], Ac / 4.0, rc[:],
                                         Alu.mult, Alu.add)
                if c0 is None:
                    eng.tensor_tensor(d[:], d[:], coef[c][:], Alu.mult)
            dm[c] = d

        # per-step tiles (groups of 1): st[c][t-1] holds step t
        st = [
            [gpool.tile([P, fc], f16, name=f"st_{c}_{t}", tag=f"st_{c}_{t}")
             for t in range(S)]
            for c, (_, fc) in enumerate(cols)
        ]

        def step_ap(c, t):
            return x0[c][:] if t == 0 else st[c][t - 1][:]

        for c in range(2):
            prologue(c)
            for t in range(1, S + 1):
                eng_of(c).tensor_tensor(step_ap(c, t), step_ap(c, t - 1),
                                        dm[c][:], Alu.subtract)

        # out-DMAs in scheduled order; group size 1 => slab == one step tile
        for c, o, g, q, off in slabs:
            fc = cols[c][1]
            assert g == 1
            dst = out[off: off + P * fc].rearrange("(p x) -> p x", p=P)
            q_map[q]().dma_start(dst, st[c][o][:])

    nc.compile()
    return nc


def kernel(features, predictions_init, W_feat, w_prob, b, attention_mask):
    preds = np.asarray(predictions_init, dtype=np.float32)
    mask = attention_mask.astype(np.float32)
    horses = mask.sum(axis=-1)                       # [B]
    c = (mask * mask) / (horses[:, None] * (V * B))  # [B,H]
    w = float(np.asarray(w_prob).reshape(-1)[0])

    c0 = float(c.flat[0])
    uniform = bool(np.all(c == c0))

    key = (w, c0 if uniform else None)
    if key not in _prog_cache:
        _prog_cache[key] = _build_program(w, c0 if uniform else None)
    nc = _prog_cache[key]

    in_maps = []
    for core in range(NCORES):
        shard = preds[core * VSH: (core + 1) * VSH].reshape(P, F)
        m = {"x0": shard.astype(np.float16)}
        if not uniform:
            ctile = np.broadcast_to(
                (c * STEP_SIZE)[None], (VSH, B, H)).reshape(P, F)
            m["coef"] = ctile.astype(np.float16)
        in_maps.append(m)

    res = run_bass_kernel_spmd(nc, in_maps, core_ids=list(range(NCORES)))

    slabs = _schedule()
    a = A_COLS
    cstart = [0, a]
    cwidth = [a, F - a]
    outs = []
    for r in res.results:
        arr = np.asarray(r["out"])
        result = np.empty((S, P, F), dtype=np.float16)
        for c, o, g, q, off in slabs:
            fc = cwidth[c]
            block = arr[off: off + P * g * fc].reshape(P, g, fc)
            result[o: o + g, :, cstart[c]: cstart[c] + fc] = (
                block.transpose(1, 0, 2)
            )
        outs.append(result.reshape(S, VSH, B, H))
    full = np.concatenate(outs, axis=1)              # [S, V, B, H]
    return np.ascontiguousarray(full[..., None].astype(np.float32))


# revision 6
# speedup vs baseline: 1.9444x; 1.0246x over previous
"""Trainium2 Bass kernel for nn_MCMCSampler.

Math: the energy gradient w.r.t. preds is purely elementwise (the feature
einsum is constant w.r.t. preds so it drops out of jax.grad):

    p     = sigmoid(x)
    grad  = c * p(1-p) * (w + beta*L),   c[b,h] = mask[b,h]/(horses[b]*V*B)
    x    <- x - STEP*grad*mask

where L = dentropy/dp collapses to x (logit o sigmoid identity, eps terms
cancel at O(eps^2)). The update is ~8e-10 per step against x ~ 0.1, so
delta is constant across the 16 steps to ~1e-16: compute delta once from
x0 and chain x_t = x_{t-1} - delta.

Precision: fp16 end to end. sigmoid'(x) = p(1-p) is evaluated via its
Taylor series 1/4 - x^2/16 (|x| <= ~0.6 here; the series error lands on
a delta that only moves the output at the 1e-9 level, far below fp16
resolution). fp16 halves DMA traffic and unlocks the DVE 2x_1p/4x_2p
perf modes. Output error vs the fp32 reference is the fp16 quantization
floor, ~2e-4 rel.

Sharding: data-parallel over V (64 variants / 8 cores), no cross-core
communication. Per-core output: [16, 128*1536] fp16 = 6.3 MB.

Schedule (CoreSim v1 cost model): a DMA occupies its issuing engine
queue (SP / ACT HWDGE, Pool SWDGE) for bytes/332GB/s; queues run
concurrently; a DMA's data is visible to the *issuing* engine's later
instructions at transfer end but to other engines only ~1.7us later
(sem path). Exploits:
  - exactly ONE column-chunk per engine, so the chain's data deps pin
    the per-engine instruction order (no compile-time round-robin
    interleave -> no head-of-line blocking);
  - Pool loads its own x0 share through its SWDGE queue and starts
    chaining ~0.6us in, vs 2.4us for the DVE share (SP load + sem);
  - per-step DMA slabs assigned to SP/ACT (and Pool once its chains
    end) by a small beam search over the measured cost constants.
"""

import numpy as np
from contextlib import ExitStack

import concourse.bass as bass
from concourse import bacc
import concourse.mybir as mybir
import concourse.tile as tile
from concourse.bass_utils import run_bass_kernel_spmd

NCORES = 8
V, B, H = 64, 1024, 24
S = 16
STEP_SIZE = 0.1
BETA = 0.1
VSH = V // NCORES          # 8 variants per core
N = VSH * B * H            # 196608 elements per core
P = 128                    # SBUF partitions
F = N // P                 # 1536 free-dim elements per partition

# --- schedule configuration (tunable) ---
A_COLS = 863               # DVE column share; Pool gets F - A_COLS
GROUPS_V = [1] * 16        # steps per DMA slab, DVE chunk
GROUPS_G = [1] * 16        # steps per DMA slab, Pool chunk
POOL_TAIL = 2              # last N Pool step slabs go out Pool's own queue

# cost constants (CoreSim v1 model, TRN2) for the static schedule
_DVE_TT = 0.5208
_DVE_TS = 0.2604
_DVE_OVH = 60.0
_POOL_OP = 0.8333
_DMA_NSB = 0.0030117
_DMA_MIN = 500.0
_SEM = 117.0

_prog_cache: dict = {}


def _schedule():
    """Static beam-search schedule. Returns slabs in emission order:
    (chunk, step_off, gsize, queue, dram_off); chunk 0 = DVE, 1 = Pool."""
    a, b = A_COLS, F - A_COLS
    # ready times of slabs
    t = 200 + max(P * a * 2 * _DMA_NSB, _DMA_MIN) + 1716.7 + _SEM
    t += 2 * (_DVE_TT * a + _DVE_OVH) + 2 * (_DVE_TS * a + _DVE_OVH)
    v_ready = []
    o = 0
    for g in GROUPS_V:
        t += g * (_DVE_TT * a + _DVE_OVH)
        v_ready.append((t, 0, o, g, P * g * a * 2))
        o += g
    t = 100 + max(P * b * 2 * _DMA_NSB, _DMA_MIN) + 3 * _POOL_OP * b
    g_ready = []
    o = 0
    for g in GROUPS_G:
        t += g * _POOL_OP * b
        g_ready.append((t, 1, o, g, P * g * b * 2))
        o += g
    # last POOL_TAIL pool slabs ride Pool's own queue (their deps force
    # them to the end of Pool's stream, so no head-of-line blocking)
    tail = g_ready[len(GROUPS_G) - POOL_TAIL:]
    slabs = sorted(v_ready + g_ready[: len(GROUPS_G) - POOL_TAIL])
    # beam search over SP/ACT assignment; state = (sp_end, act_end)
    sp0 = 200 + max(P * a * 2 * _DMA_NSB, _DMA_MIN)
    states = {(sp0, 200.0): ()}
    for r, c, o, g, nb in slabs:
        dur = max(nb * _DMA_NSB, _DMA_MIN)
        new = {}
        for (sp, act), hist in states.items():
            for qi, qv in ((0, sp), (1, act)):
                end = max(qv, r + _SEM) + dur
                ns = (end, act) if qi == 0 else (sp, end)
                if ns not in new:
                    new[ns] = hist + (qi,)
        states = dict(sorted(new.items(), key=lambda kv: max(kv[0]))[:600])
    best = min(states.items(), key=lambda kv: max(kv[0]))
    qnames = ('s', 'a')
    assigned = [(r, c, o, g, nb, qnames[qi])
                for (r, c, o, g, nb), qi in zip(slabs, best[1])]
    assigned += [(r, c, o, g, nb, 'g') for r, c, o, g, nb in tail]
    res = []
    off = 0
    for r, c, o, g, nb, q in assigned:
        res.append((c, o, g, q, off))
        off += nb // 2
    assert off == S * P * F
    return res


def _build_program(w: float, c0: float | None):
    """c0: uniform coefficient, or None -> per-element coef input."""
    nc = bacc.Bacc("TRN2", target_bir_lowering=False, debug=False)
    f16 = mybir.dt.float16
    Alu = mybir.AluOpType

    x_in = nc.declare_dram_parameter("x0", [P, F], f16, isOutput=False)
    coef_in = None
    if c0 is None:
        coef_in = nc.declare_dram_parameter("coef", [P, F], f16, isOutput=False)
    out = nc.declare_dram_parameter("out", [S * P * F], f16, isOutput=True)

    slabs = _schedule()
    a, b = A_COLS, F - A_COLS
    cols = [(0, a), (a, b)]            # (col0, fc) per chunk
    engs = ['v', 'g']
    cs = STEP_SIZE * (c0 if c0 is not None else 1.0)
    Ac = float(BETA * cs)
    Bc = float(w * cs)

    with ExitStack() as ctx:
        tc = ctx.enter_context(tile.TileContext(nc))
        cpool = ctx.enter_context(tc.tile_pool(name="const", bufs=1))
        gpool = ctx.enter_context(tc.tile_pool(name="groups", bufs=1))

        q_map = {'s': lambda: nc.sync, 'a': lambda: nc.scalar,
                 'g': lambda: nc.gpsimd}

        def eng_of(c):
            return nc.vector if engs[c] == 'v' else nc.gpsimd

        # inputs: DVE share via SP, Pool share via Pool's own SWDGE queue
        x0 = []
        for c, (c0_, fc) in enumerate(cols):
            t = cpool.tile([P, fc], f16, name=f"x0_{c}", tag=f"x0_{c}")
            q_map['s' if engs[c] == 'v' else 'g']().dma_start(
                t[:], x_in[:, c0_: c0_ + fc])
            x0.append(t)
        coef = []
        if c0 is None:
            for c, (c0_, fc) in enumerate(cols):
                t = cpool.tile([P, fc], f16, name=f"cf_{c}", tag=f"cf_{c}")
                q_map['a']().dma_start(t[:], coef_in[:, c0_: c0_ + fc])
                coef.append(t)

        dm = [None, None]

        def prologue(c):
            fc = cols[c][1]
            eng = eng_of(c)
            t2 = cpool.tile([P, fc], f16, name=f"t2_{c}", tag=f"t2_{c}")
            eng.tensor_tensor(t2[:], x0[c][:], x0[c][:], Alu.mult)
            d = cpool.tile([P, fc], f16, name=f"d_{c}", tag=f"d_{c}")
            if engs[c] == 'v':
                # delta = (1/4 - x^2/16) * (A*x + B), exact product form
                sc = cpool.tile([P, fc], f16, name=f"s_{c}", tag=f"s_{c}")
                eng.tensor_scalar(sc[:], t2[:], -1.0 / 16.0, 0.25,
                                  Alu.mult, Alu.add)
                uc = cpool.tile([P, fc], f16, name=f"u_{c}", tag=f"u_{c}")
                eng.tensor_scalar(uc[:], x0[c][:], Ac, Bc, Alu.mult, Alu.add)
                if c0 is None:
                    eng.tensor_tensor(uc[:], uc[:], coef[c][:], Alu.mult)
                eng.tensor_tensor(d[:], sc[:], uc[:], Alu.mult)
            else:
                # delta ~= B/4 + (A/4)x - (B/16)x^2 (x^3 term is ~1% of a
                # delta that is itself 1e-9 against fp16's 6e-5 resolution)
                rc = cpool.tile([P, fc], f16, name=f"r_{c}", tag=f"r_{c}")
                eng.tensor_scalar(rc[:], t2[:], -Bc / 16.0, Bc / 4.0,
                                  Alu.mult, Alu.add)
                eng.scalar_tensor_tensor(d[:], x0[c][:], Ac / 4.0, rc[:],
                                         Alu.mult, Alu.add)
                if c0 is None:
                    eng.tensor_tensor(d[:], d[:], coef[c][:], Alu.mult)
            dm[c] = d

        # per-step tiles (groups of 1): st[c][t-1] holds step t
        st = [
            [gpool.tile([P, fc], f16, name=f"st_{c}_{t}", tag=f"st_{c}_{t}")
             for t in range(S)]
            for c, (_, fc) in enumerate(cols)
        ]

        def step_ap(c, t):
            return x0[c][:] if t == 0 else st[c][t - 1][:]

        for c in range(2):
            prologue(c)
            for t in range(1, S + 1):
                eng_of(c).tensor_tensor(step_ap(c, t), step_ap(c, t - 1),
                                        dm[c][:], Alu.subtract)

        # out-DMAs in scheduled order; group size 1 => slab == one step tile
        for c, o, g, q, off in slabs:
            fc = cols[c][1]
            assert g == 1
            dst = out[off: off + P * fc].rearrange("(p x) -> p x", p=P)
            q_map[q]().dma_start(dst, st[c][o][:])

    nc.compile()
    return nc


def kernel(features, predictions_init, W_feat, w_prob, b, attention_mask):
    preds = np.asarray(predictions_init, dtype=np.float32)
    mask = attention_mask.astype(np.float32)
    horses = mask.sum(axis=-1)                       # [B]
    c = (mask * mask) / (horses[:, None] * (V * B))  # [B,H]
    w = float(np.asarray(w_prob).reshape(-1)[0])

    c0 = float(c.flat[0])
    uniform = bool(np.all(c == c0))

    key = (w, c0 if uniform else None)
    if key not in _prog_cache:
        _prog_cache[key] = _build_program(w, c0 if uniform else None)
    nc = _prog_cache[key]

    in_maps = []
    for core in range(NCORES):
        shard = preds[core * VSH: (core + 1) * VSH].reshape(P, F)
        m = {"x0": shard.astype(np.float16)}
        if not uniform:
            ctile = np.broadcast_to(
                (c * STEP_SIZE)[None], (VSH, B, H)).reshape(P, F)
            m["coef"] = ctile.astype(np.float16)
        in_maps.append(m)

    res = run_bass_kernel_spmd(nc, in_maps, core_ids=list(range(NCORES)))

    slabs = _schedule()
    a = A_COLS
    cstart = [0, a]
    cwidth = [a, F - a]
    outs = []
    for r in res.results:
        arr = np.asarray(r["out"])
        result = np.empty((S, P, F), dtype=np.float16)
        for c, o, g, q, off in slabs:
            fc = cwidth[c]
            block = arr[off: off + P * g * fc].reshape(P, g, fc)
            result[o: o + g, :, cstart[c]: cstart[c] + fc] = (
                block.transpose(1, 0, 2)
            )
        outs.append(result.reshape(S, VSH, B, H))
    full = np.concatenate(outs, axis=1)              # [S, V, B, H]
    return np.ascontiguousarray(full[..., None].astype(np.float32))


# revision 7
# speedup vs baseline: 1.9588x; 1.0074x over previous
"""Trainium2 Bass kernel for nn_MCMCSampler.

Math: the energy gradient w.r.t. preds is purely elementwise (the feature
einsum is constant w.r.t. preds so it drops out of jax.grad):

    p     = sigmoid(x)
    grad  = c * p(1-p) * (w + beta*L),   c[b,h] = mask[b,h]/(horses[b]*V*B)
    x    <- x - STEP*grad*mask

where L = dentropy/dp collapses to x (logit o sigmoid identity, eps terms
cancel at O(eps^2)). The update is ~8e-10 per step against x ~ 0.1, so
delta is constant across the 16 steps to ~1e-16: compute delta once from
x0 and chain x_t = x_{t-1} - delta.

Precision: fp16 end to end. sigmoid'(x) = p(1-p) is evaluated via its
Taylor series 1/4 - x^2/16 (|x| <= ~0.6 here; the series error lands on
a delta that only moves the output at the 1e-9 level, far below fp16
resolution). fp16 halves DMA traffic and unlocks the DVE 2x_1p/4x_2p
perf modes. Output error vs the fp32 reference is the fp16 quantization
floor, ~2e-4 rel.

Sharding: data-parallel over V (64 variants / 8 cores), no cross-core
communication. Per-core output: [16, 128*1536] fp16 = 6.3 MB.

Schedule (CoreSim v1 cost model): a DMA occupies its issuing engine
queue (SP / ACT HWDGE, Pool SWDGE) for bytes/332GB/s; queues run
concurrently; a DMA's data is visible to the *issuing* engine's later
instructions at transfer end but to other engines only ~1.7us later
(sem path). Exploits:
  - exactly ONE column-chunk per engine, so the chain's data deps pin
    the per-engine instruction order (no compile-time round-robin
    interleave -> no head-of-line blocking);
  - Pool loads its own x0 share through its SWDGE queue and starts
    chaining ~0.6us in, vs 2.4us for the DVE share (SP load + sem);
  - per-step DMA slabs assigned to SP/ACT (and Pool once its chains
    end) by a small beam search over the measured cost constants.
"""

import numpy as np
from contextlib import ExitStack

import concourse.bass as bass
from concourse import bacc
import concourse.mybir as mybir
import concourse.tile as tile
from concourse.bass_utils import run_bass_kernel_spmd

NCORES = 8
V, B, H = 64, 1024, 24
S = 16
STEP_SIZE = 0.1
BETA = 0.1
VSH = V // NCORES          # 8 variants per core
N = VSH * B * H            # 196608 elements per core
P = 128                    # SBUF partitions
F = N // P                 # 1536 free-dim elements per partition

# --- schedule configuration (tunable) ---
A_COLS = 821               # DVE column share; Pool gets F - A_COLS
GROUPS_V = [1] * 16        # steps per DMA slab, DVE chunk
GROUPS_G = [1] * 16        # steps per DMA slab, Pool chunk
POOL_TAIL = 2              # last N Pool step slabs go out Pool's own queue

# cost constants (CoreSim v1 model, TRN2) for the static schedule
_DVE_TT = 0.5208
_DVE_TS = 0.2604
_DVE_OVH = 60.0
_POOL_OP = 0.8333
_DMA_NSB = 0.0030117
_DMA_MIN = 500.0
_SEM = 250.0

_prog_cache: dict = {}


def _schedule():
    """Static beam-search schedule. Returns slabs in emission order:
    (chunk, step_off, gsize, queue, dram_off); chunk 0 = DVE, 1 = Pool."""
    a, b = A_COLS, F - A_COLS
    # ready times of slabs
    t = 200 + max(P * a * 2 * _DMA_NSB, _DMA_MIN) + 1716.7 + _SEM
    t += 2 * (_DVE_TT * a + _DVE_OVH) + 2 * (_DVE_TS * a + _DVE_OVH)
    v_ready = []
    o = 0
    for g in GROUPS_V:
        t += g * (_DVE_TT * a + _DVE_OVH)
        v_ready.append((t, 0, o, g, P * g * a * 2))
        o += g
    t = 100 + max(P * b * 2 * _DMA_NSB, _DMA_MIN) + 3 * _POOL_OP * b
    g_ready = []
    o = 0
    for g in GROUPS_G:
        t += g * _POOL_OP * b
        g_ready.append((t, 1, o, g, P * g * b * 2))
        o += g
    # last POOL_TAIL pool slabs ride Pool's own queue (their deps force
    # them to the end of Pool's stream, so no head-of-line blocking)
    tail = g_ready[len(GROUPS_G) - POOL_TAIL:]
    slabs = sorted(v_ready + g_ready[: len(GROUPS_G) - POOL_TAIL])
    # beam search over SP/ACT assignment; state = (sp_end, act_end)
    sp0 = 200 + max(P * a * 2 * _DMA_NSB, _DMA_MIN)
    states = {(sp0, 200.0): ()}
    for r, c, o, g, nb in slabs:
        dur = max(nb * _DMA_NSB, _DMA_MIN)
        new = {}
        for (sp, act), hist in states.items():
            for qi, qv in ((0, sp), (1, act)):
                end = max(qv, r + _SEM) + dur
                ns = (end, act) if qi == 0 else (sp, end)
                if ns not in new:
                    new[ns] = hist + (qi,)
        states = dict(sorted(new.items(), key=lambda kv: max(kv[0]))[:600])
    best = min(states.items(), key=lambda kv: max(kv[0]))
    qnames = ('s', 'a')
    assigned = [(r, c, o, g, nb, qnames[qi])
                for (r, c, o, g, nb), qi in zip(slabs, best[1])]
    assigned += [(r, c, o, g, nb, 'g') for r, c, o, g, nb in tail]
    res = []
    off = 0
    for r, c, o, g, nb, q in assigned:
        res.append((c, o, g, q, off))
        off += nb // 2
    assert off == S * P * F
    return res


def _build_program(w: float, c0: float | None):
    """c0: uniform coefficient, or None -> per-element coef input."""
    nc = bacc.Bacc("TRN2", target_bir_lowering=False, debug=False)
    f16 = mybir.dt.float16
    Alu = mybir.AluOpType

    x_in = nc.declare_dram_parameter("x0", [P, F], f16, isOutput=False)
    coef_in = None
    if c0 is None:
        coef_in = nc.declare_dram_parameter("coef", [P, F], f16, isOutput=False)
    out = nc.declare_dram_parameter("out", [S * P * F], f16, isOutput=True)

    slabs = _schedule()
    a, b = A_COLS, F - A_COLS
    cols = [(0, a), (a, b)]            # (col0, fc) per chunk
    engs = ['v', 'g']
    cs = STEP_SIZE * (c0 if c0 is not None else 1.0)
    Ac = float(BETA * cs)
    Bc = float(w * cs)

    with ExitStack() as ctx:
        tc = ctx.enter_context(tile.TileContext(nc))
        cpool = ctx.enter_context(tc.tile_pool(name="const", bufs=1))
        gpool = ctx.enter_context(tc.tile_pool(name="groups", bufs=1))

        q_map = {'s': lambda: nc.sync, 'a': lambda: nc.scalar,
                 'g': lambda: nc.gpsimd}

        def eng_of(c):
            return nc.vector if engs[c] == 'v' else nc.gpsimd

        # inputs: DVE share via SP, Pool share via Pool's own SWDGE queue
        x0 = []
        for c, (c0_, fc) in enumerate(cols):
            t = cpool.tile([P, fc], f16, name=f"x0_{c}", tag=f"x0_{c}")
            q_map['s' if engs[c] == 'v' else 'g']().dma_start(
                t[:], x_in[:, c0_: c0_ + fc])
            x0.append(t)
        coef = []
        if c0 is None:
            for c, (c0_, fc) in enumerate(cols):
                t = cpool.tile([P, fc], f16, name=f"cf_{c}", tag=f"cf_{c}")
                q_map['a']().dma_start(t[:], coef_in[:, c0_: c0_ + fc])
                coef.append(t)

        dm = [None, None]

        def prologue(c):
            fc = cols[c][1]
            eng = eng_of(c)
            t2 = cpool.tile([P, fc], f16, name=f"t2_{c}", tag=f"t2_{c}")
            eng.tensor_tensor(t2[:], x0[c][:], x0[c][:], Alu.mult)
            d = cpool.tile([P, fc], f16, name=f"d_{c}", tag=f"d_{c}")
            if engs[c] == 'v':
                # delta = (1/4 - x^2/16) * (A*x + B), exact product form
                sc = cpool.tile([P, fc], f16, name=f"s_{c}", tag=f"s_{c}")
                eng.tensor_scalar(sc[:], t2[:], -1.0 / 16.0, 0.25,
                                  Alu.mult, Alu.add)
                uc = cpool.tile([P, fc], f16, name=f"u_{c}", tag=f"u_{c}")
                eng.tensor_scalar(uc[:], x0[c][:], Ac, Bc, Alu.mult, Alu.add)
                if c0 is None:
                    eng.tensor_tensor(uc[:], uc[:], coef[c][:], Alu.mult)
                eng.tensor_tensor(d[:], sc[:], uc[:], Alu.mult)
            else:
                # delta ~= B/4 + (A/4)x - (B/16)x^2 (x^3 term is ~1% of a
                # delta that is itself 1e-9 against fp16's 6e-5 resolution)
                rc = cpool.tile([P, fc], f16, name=f"r_{c}", tag=f"r_{c}")
                eng.tensor_scalar(rc[:], t2[:], -Bc / 16.0, Bc / 4.0,
                                  Alu.mult, Alu.add)
                eng.scalar_tensor_tensor(d[:], x0[c][:], Ac / 4.0, rc[:],
                                         Alu.mult, Alu.add)
                if c0 is None:
                    eng.tensor_tensor(d[:], d[:], coef[c][:], Alu.mult)
            dm[c] = d

        # per-step tiles (groups of 1): st[c][t-1] holds step t
        st = [
            [gpool.tile([P, fc], f16, name=f"st_{c}_{t}", tag=f"st_{c}_{t}")
             for t in range(S)]
            for c, (_, fc) in enumerate(cols)
        ]

        def step_ap(c, t):
            return x0[c][:] if t == 0 else st[c][t - 1][:]

        for c in range(2):
            prologue(c)
            for t in range(1, S + 1):
                eng_of(c).tensor_tensor(step_ap(c, t), step_ap(c, t - 1),
                                        dm[c][:], Alu.subtract)

        # out-DMAs in scheduled order; group size 1 => slab == one step tile
        for c, o, g, q, off in slabs:
            fc = cols[c][1]
            assert g == 1
            dst = out[off: off + P * fc].rearrange("(p x) -> p x", p=P)
            q_map[q]().dma_start(dst, st[c][o][:])

    nc.compile()
    return nc


def kernel(features, predictions_init, W_feat, w_prob, b, attention_mask):
    preds = np.asarray(predictions_init, dtype=np.float32)
    mask = attention_mask.astype(np.float32)
    horses = mask.sum(axis=-1)                       # [B]
    c = (mask * mask) / (horses[:, None] * (V * B))  # [B,H]
    w = float(np.asarray(w_prob).reshape(-1)[0])

    c0 = float(c.flat[0])
    uniform = bool(np.all(c == c0))

    key = (w, c0 if uniform else None)
    if key not in _prog_cache:
        _prog_cache[key] = _build_program(w, c0 if uniform else None)
    nc = _prog_cache[key]

    in_maps = []
    for core in range(NCORES):
        shard = preds[core * VSH: (core + 1) * VSH].reshape(P, F)
        m = {"x0": shard.astype(np.float16)}
        if not uniform:
            ctile = np.broadcast_to(
                (c * STEP_SIZE)[None], (VSH, B, H)).reshape(P, F)
            m["coef"] = ctile.astype(np.float16)
        in_maps.append(m)

    res = run_bass_kernel_spmd(nc, in_maps, core_ids=list(range(NCORES)))

    slabs = _schedule()
    a = A_COLS
    cstart = [0, a]
    cwidth = [a, F - a]
    outs = []
    for r in res.results:
        arr = np.asarray(r["out"])
        result = np.empty((S, P, F), dtype=np.float16)
        for c, o, g, q, off in slabs:
            fc = cwidth[c]
            block = arr[off: off + P * g * fc].reshape(P, g, fc)
            result[o: o + g, :, cstart[c]: cstart[c] + fc] = (
                block.transpose(1, 0, 2)
            )
        outs.append(result.reshape(S, VSH, B, H))
    full = np.concatenate(outs, axis=1)              # [S, V, B, H]
    return np.ascontiguousarray(full[..., None].astype(np.float32))
